# revision 28
# baseline (speedup 1.0000x reference)
"""Trainium2 Bass kernel for MinimalKAN forward (nn_MinimalKAN_Normalized).

Math:
  a = sigmoid(alpha)
  out = (1-a) * (x @ W.T + b) + (a/sqrt(I)) * (x @ C0 + x^2 @ C1 + x^3 @ C2)

Folding the alpha blend into the weights on the host gives exactly
  out = x @ A1 + x^2 @ A2 + x^3 @ A3 + b_eff
with A1 = (1-a) W.T + s C0, A2 = s C1, A3 = s C2, b_eff = (1-a) b,
s = a/sqrt(I).

Device strategy (data-parallel over batch, 8 cores), per core shard 4096
rows; x^T is fed per core so the contraction index sits on SBUF partitions.
All weights are host-scaled by S=4096 (power of two => exact in fp16/fp8
scale space) and one PSUM unscale folds 1/S into the bias add; device
output is fp16 (host upcasts; adds ~2e-4 rel err), halving out traffic.

Two schemes (KAN_SCHEME):
  fp8 (default): every matmul is an fp8e4 DoubleRow (2 K-tiles, 0.5 cyc/row).
    The linear term is compensated quantization:
      x @ A1 ~= x8 @ A1q + x8 @ Rq + (64 dx)8 @ (A1*64)q
    with x8 = e4m3(x), dx = x - x8 (host-scaled by 64 to clear e4m3
    subnormals; its weight copy is scaled 1/64 so the PSUM scale is
    uniform), Rq = e4m3(A1*S - A1q).  Basis x2/x3 are computed on device
    from x8 (ACT Square -> fp8, DVE mult -> fp8); their e4m3 error is
    damped by s*0.01.  Host-measured rel err ~9.2e-3 (threshold 2e-2).
    Per 128-row tile: 10 DR matmuls = 2560 PE cycles.
  mixed: linear term in fp16 (4 matmuls) + 4 DR basis matmuls = 3072 PE
    cycles per tile; rel err ~5.0e-3.  Fallback if DoubleRow underperforms
    the cost model on hardware.

Per 512-row batch group the emission interleaves so PE never starves:
linear block j+1 is emitted between basis block j; PSUM->SBUF unscale+bias
alternates DVE (j0/j3) and Pool (j1/j2); out DMA rides the SP ring; weights
ride the ACT ring; group 0 runs k-outer warmup so matmuls start as soon as
the first xT k-slice lands.
"""

import os
import numpy as np

import concourse.bass as bass
from concourse import bacc
import concourse.mybir as mybir
import concourse.tile as tile
from concourse.bass_utils import run_bass_kernel_spmd

N_CORES = 8
B, I, O = 32768, 512, 512
BS = B // N_CORES          # rows per core
P = 128
N_TILES = BS // P          # 32 tiles per core
KS = I // P                # 4 contraction slices per basis

_SCHEME = os.environ.get("KAN_SCHEME", "mixed")   # mixed | fp8
_GROUP = int(os.environ.get("KAN_GROUP", "4"))

WSCALE = 4096.0  # host multiplies weights by this; PSUM is unscaled once
DSCALE = 64.0    # host multiplies the x residual by this (e4m3 subnormals)


def _build(scheme: str = _SCHEME, repeat: int = 1,
           group: int = _GROUP) -> bass.Bass:
    G = group
    GB = G * P                     # batch rows per group
    n_groups = N_TILES // G
    sq = mybir.ActivationFunctionType.Square
    f16 = mybir.dt.float16
    f8 = mybir.dt.float8e4
    f32 = mybir.dt.float32
    DR = mybir.MatmulPerfMode.DoubleRow

    nc = bacc.Bacc("TRN2", target_bir_lowering=False, debug=False,
                   num_devices=N_CORES)

    fp8 = scheme == "fp8"
    if fp8:
        # x8/d8 interleaved by k-pair: slices [x k0,k1 | d k0,k1 | x k2,k3
        # | d k2,k3] so one DMA per group feeds everything and warmup can
        # split the group load in halves
        x_d = nc.dram_tensor("xd", [2 * I, BS], f8, kind="ExternalInput")
        x_r = x_d.rearrange("(ks p) b -> p ks b", p=P)
        # slices [A1q,Rq,A1d @ k0,k1 | A1q,Rq,A1d @ k2,k3 | A2q | A3q]
        w8_d = nc.dram_tensor("w8", [5 * I, O], f8, kind="ExternalInput")
        w8_r = w8_d.rearrange("(ws p) o -> p ws o", p=P)
        NW8 = 5 * KS
    else:
        x_d = nc.dram_tensor("xt", [I, BS], f16, kind="ExternalInput")
        x_r = x_d.rearrange("(ks p) b -> p ks b", p=P)
        w16_d = nc.dram_tensor("w16", [I, O], f16, kind="ExternalInput")
        w16_r = w16_d.rearrange("(ks p) o -> p ks o", p=P)
        w8_d = nc.dram_tensor("w8", [2 * I, O], f8, kind="ExternalInput")
        w8_r = w8_d.rearrange("(ws p) o -> p ws o", p=P)
        NW8 = 2 * KS
    b_d = nc.dram_tensor("bias", [P, O], f16, kind="ExternalInput")
    o_d = nc.dram_tensor("out", [BS, O], f16, kind="ExternalOutput")
    o_t = o_d.rearrange("(t p) k -> t p k", p=P)
    o_g = o_d.rearrange("(g a p) k -> g p a k", a=G, p=P)

    with tile.TileContext(nc) as tc:
        with (
            tc.tile_pool(name="const", bufs=1) as const,
            tc.tile_pool(name="xt", bufs=4) as xt,
            tc.tile_pool(name="outp", bufs=4) as outp,
            tc.tile_pool(name="psum_o", bufs=8, space="PSUM") as psum_o,
        ):
            # weights on the ACT ring so the first xT group (SP ring) is not
            # queued behind them
            if not fp8:
                # k0 slice first: the warmup k-outer matmuls only need it
                # plus the first xT group
                w16sb = const.tile([P, KS, O], f16)
                nc.scalar.dma_start(w16sb[:, 0, :], w16_r[:, 0, :])
                nc.scalar.dma_start(w16sb[:, 1:KS, :], w16_r[:, 1:KS, :])
            w8sb = const.tile([P, NW8, O], f8)
            if fp8:
                # three chunks: warmup kp0 lin / kp1 lin / basis weights
                nc.scalar.dma_start(w8sb[:, 0:6, :], w8_r[:, 0:6, :])
                nc.scalar.dma_start(w8sb[:, 6:12, :], w8_r[:, 6:12, :])
                nc.scalar.dma_start(w8sb[:, 12:NW8, :], w8_r[:, 12:NW8, :])
            else:
                nc.scalar.dma_start(w8sb[:], w8_r[:, :, :])
            bsb = const.tile([P, O], f16)
            nc.scalar.dma_start(bsb[:], b_d[:, :])

            def body():
                first = True
                for g in [i for _ in range(max(repeat, 1))
                          for i in range(n_groups)]:
                    cols = slice(g * GB, (g + 1) * GB)
                    last_g = g == n_groups - 1
                    if fp8:
                        xd = xt.tile([P, 2 * KS, GB], f8, tag="xd")
                        if first:
                            # halves: warmup kp0 runs after the first half
                            nc.sync.dma_start(xd[:, 0:KS, :],
                                              x_r[:, 0:KS, cols])
                            nc.sync.dma_start(xd[:, KS:2 * KS, :],
                                              x_r[:, KS:2 * KS, cols])
                        else:
                            nc.sync.dma_start(xd[:], x_r[:, :, cols])

                        def xpair(kp):
                            return xd[:, 4 * kp:4 * kp + 2, :]

                        def dpair(kp):
                            return xd[:, 4 * kp + 2:4 * kp + 4, :]
                    else:
                        xT = xt.tile([P, KS, GB], f16, tag="xT")
                        if first:
                            for k in range(KS):
                                nc.sync.dma_start(xT[:, k, :],
                                                  x_r[:, k, cols])
                        else:
                            nc.sync.dma_start(xT[:], x_r[:, :, cols])

                    x2T = xt.tile([P, KS, GB], f8, tag="x2T")
                    x3T = xt.tile([P, KS, GB], f8, tag="x3T")
                    o_sb = outp.tile([P, G, O], f16, tag="o_sb")
                    jsl = [slice(j * P, (j + 1) * P) for j in range(G)]
                    # basis values early: ACT squares, DVE cubes, so the
                    # basis matmuls (one-tile lag) never starve
                    for j in range(G):
                        if fp8 and first:
                            # warmup: per k-pair, following the half DMAs
                            for kp in range(KS // 2):
                                xp = xpair(kp)[:, :, jsl[j]]
                                x2p = x2T[:, 2 * kp:2 * kp + 2, jsl[j]]
                                nc.scalar.activation(x2p, xp, sq)
                                nc.vector.tensor_mul(
                                    x3T[:, 2 * kp:2 * kp + 2, jsl[j]],
                                    x2p, xp)
                        elif fp8:
                            # strided view of the x slices of xd, k-order
                            # matching x2T/x3T's contiguous k dim
                            xsel = xd[:].rearrange(
                                "p (kp sel i) b -> p kp sel i b",
                                kp=2, sel=2, i=2)[:, :, 0, :, jsl[j]]
                            nc.scalar.activation(x2T[:, :, jsl[j]], xsel,
                                                 sq)
                            nc.vector.tensor_mul(x3T[:, :, jsl[j]],
                                                 x2T[:, :, jsl[j]], xsel)
                        else:
                            nc.scalar.activation(x2T[:, :, jsl[j]],
                                                 xT[:, :, jsl[j]], sq)
                            nc.vector.tensor_mul(x3T[:, :, jsl[j]],
                                                 x2T[:, :, jsl[j]],
                                                 xT[:, :, jsl[j]])
                    n_mm = 10 if fp8 else 2 * KS
                    pos = [psum_o.tile([P, O], f32, tag="po", name="po")
                           for _ in range(G)]
                    idxs = [0] * G

                    def lin_mm(j, kp, wp):
                        # wp: 0=A1q(x8) 1=Rq(x8) 2=A1d(d8)
                        lhsT = (dpair(kp) if wp == 2
                                else xpair(kp))[:, :, jsl[j]]
                        ws = kp * 6 + wp * 2
                        nc.tensor.matmul(
                            pos[j][:], lhsT, w8sb[:, ws:ws + 2, :],
                            perf_mode=DR, start=(idxs[j] == 0),
                            stop=(idxs[j] == n_mm - 1),
                            skip_group_check=True)
                        idxs[j] += 1

                    def lin(j):
                        if fp8:
                            for kp in range(KS // 2):
                                for wp in range(3):
                                    lin_mm(j, kp, wp)
                        else:
                            for k in range(KS):
                                nc.tensor.matmul(
                                    pos[j][:], xT[:, k, jsl[j]],
                                    w16sb[:, k, :], start=(idxs[j] == 0),
                                    stop=(idxs[j] == n_mm - 1),
                                    skip_group_check=True)
                                idxs[j] += 1

                    def basis(j):
                        for bi, XT in enumerate((x2T, x3T)):
                            base = (12 if fp8 else 0) + 4 * bi
                            for kp in range(KS // 2):
                                nc.tensor.matmul(
                                    pos[j][:],
                                    XT[:, 2 * kp:2 * kp + 2, jsl[j]],
                                    w8sb[:, base + 2 * kp:
                                         base + 2 * kp + 2, :],
                                    perf_mode=DR, start=(idxs[j] == 0),
                                    stop=(idxs[j] == n_mm - 1),
                                    skip_group_check=True)
                                idxs[j] += 1

                    def finish(j):
                        # PSUM -> SBUF with unscale+bias on DVE (GPSIMD
                        # cannot read PSUM: BIR verifier rejects it)
                        eng = nc.vector
                        eng.scalar_tensor_tensor(
                            o_sb[:, j, :], pos[j][:], 1.0 / WSCALE, bsb[:],
                            mybir.AluOpType.mult, mybir.AluOpType.add)
                        if last_g:
                            # per-tile DMAs so the drain tail is short
                            nc.sync.dma_start(o_t[g * G + j],
                                              o_sb[:, j, :])
                        elif j == G - 1:
                            nc.sync.dma_start(o_g[g], o_sb[:])

                    if first:
                        # warmup group: k-outer so matmuls start as soon as
                        # the first xd half + first weight chunk arrive
                        if fp8:
                            for kp in range(KS // 2):
                                for wp in range(3):
                                    for j in range(G):
                                        lin_mm(j, kp, wp)
                        else:
                            for k in range(KS):
                                for j in range(G):
                                    nc.tensor.matmul(
                                        pos[j][:], xT[:, k, jsl[j]],
                                        w16sb[:, k, :],
                                        start=(idxs[j] == 0), stop=False,
                                        skip_group_check=True)
                                    idxs[j] += 1
                        for j in range(G):
                            basis(j)
                            finish(j)
                        first = False
                    else:
                        # steady state: lin block j+1 runs while basis j
                        # finishes, so the basis matmuls never starve
                        lin(0)
                        for j in range(G):
                            if j + 1 < G:
                                lin(j + 1)
                            basis(j)
                            finish(j)

            body()

    nc.compile()
    return nc


_NC_CACHE: dict[str, bass.Bass] = {}


def _get_nc() -> bass.Bass:
    nc = _NC_CACHE.get(_SCHEME)
    if nc is None:
        nc = _build(_SCHEME)
        _NC_CACHE[_SCHEME] = nc
    return nc


def _fold_weights(coeffs, W, b, alpha):
    a = 1.0 / (1.0 + np.exp(-np.float64(alpha)))
    s = a / np.sqrt(np.float64(I))
    A1 = ((1.0 - a) * W.astype(np.float64).T
          + s * coeffs[:, :, 0].astype(np.float64)).astype(np.float32)
    A2 = (s * coeffs[:, :, 1].astype(np.float64)).astype(np.float32)
    A3 = (s * coeffs[:, :, 2].astype(np.float64)).astype(np.float32)
    b_eff = ((1.0 - a) * b.astype(np.float64)).astype(np.float32)
    bias_rep = np.ascontiguousarray(
        np.broadcast_to(b_eff[None, :], (P, O)).astype(np.float32))
    return A1, A2, A3, bias_rep


def _make_in_maps(x, coeffs, W, b, alpha):
    import ml_dtypes
    e4 = ml_dtypes.float8_e4m3
    A1, A2, A3, bias_rep = _fold_weights(coeffs, W, b, alpha)
    x = np.asarray(x, dtype=np.float32)
    common = {"bias": bias_rep.astype(np.float16)}
    if _SCHEME == "fp8":
        A1s = A1 * WSCALE
        A1q = A1s.astype(e4)
        Rq = (A1s - A1q.astype(np.float32)).astype(e4)
        A1d = (A1 * (WSCALE / DSCALE)).astype(e4)
        A2q = (A2 * WSCALE).astype(e4)
        A3q = (A3 * WSCALE).astype(e4)
        # device slice order: [A1q,Rq,A1d @ k0,k1 | A1q,Rq,A1d @ k2,k3 |
        # A2q | A3q], each k-slice is 128 rows
        H = 2 * P
        parts = []
        for kp in range(2):
            sl = slice(kp * H, (kp + 1) * H)
            parts += [A1q[sl], Rq[sl], A1d[sl]]
        parts += [A2q, A3q]
        common["w8"] = np.ascontiguousarray(np.concatenate(parts, axis=0))
    else:
        common["w16"] = np.ascontiguousarray(
            (A1 * WSCALE).astype(np.float16))
        common["w8"] = np.ascontiguousarray(np.concatenate(
            [A2 * WSCALE, A3 * WSCALE], axis=0).astype(e4))
    in_maps = []
    for c in range(N_CORES):
        shard_t = np.ascontiguousarray(x[c * BS:(c + 1) * BS].T)
        m = dict(common)
        if _SCHEME == "fp8":
            x8 = shard_t.astype(e4)
            d8 = ((shard_t - x8.astype(np.float32)) * DSCALE).astype(e4)
            # interleave by k-pair: [x k0,k1 | d k0,k1 | x k2,k3 | d k2,k3]
            H = 2 * P
            m["xd"] = np.ascontiguousarray(np.concatenate(
                [x8[0:H], d8[0:H], x8[H:2 * H], d8[H:2 * H]], axis=0))
        else:
            m["xt"] = shard_t.astype(np.float16)
        in_maps.append(m)
    return in_maps


def _run(x, coeffs, W, b, alpha, trace=False):
    nc = _get_nc()
    in_maps = _make_in_maps(x, coeffs, W, b, alpha)
    res = run_bass_kernel_spmd(nc, in_maps, core_ids=list(range(N_CORES)),
                               trace=trace)
    out = np.concatenate([np.asarray(r["out"], dtype=np.float32)
                          for r in res.results], axis=0)
    return out, res


def kernel(x, coeffs, W, b, alpha):
    out, _ = _run(x, coeffs, W, b, alpha, trace=False)
    return out
